# revision 15
# baseline (speedup 1.0000x reference)
"""GRU layer kernel for Trainium2, data-parallel over 8 NeuronCores.

Strategy (feature-major / weight-stationary):
  - Shard batch N=2048 -> 8 cores x NB=256.
  - On host: transpose inputs to feature-major xT[t] = [D, NB] per core, and
    pre-pack each weight matrix W[dout,din] into the PE lhsT tile layout
    (tile (k,m): lhsT[p, q] = W[m*128+q, k*128+p]).
  - On chip per timestep (all matmuls fp32r, PSUM fp32 accumulate):
      z_pre.T = Wz_x| x_t.T  +  Wz_h| h.T      (8 K-tiles into PSUM)
      r_pre.T = likewise
      g_pre.T = Wh_x| x_t.T  +  Wh_h| (r*h).T
      z,r = sigmoid(+bias) on ACT; g = tanh(+bias); blend on DVE.
    x-projection matmuls of step t+1 have no dependency on step t and fill
    the PE pipeline while ACT/DVE resolve the recurrence.
  - Output written feature-major [T, D, NB] per core; host transposes back.
"""
import os
import numpy as np

N, D = 2048, 512
T = int(os.environ.get("GRU_T", "64"))
NC = 8
NB = N // NC          # 256 batch rows per core
KT = D // 128         # 4 k-tiles
MT = D // 128         # 4 m-tiles

MM_DT = os.environ.get("GRU_MM_DT", "bf16")    # bf16 | fp32r | fp32

_CACHE = {}
LAST_RESULT = None


def _build_nc():
    import concourse.bacc as bacc
    import concourse.mybir as mybir
    from concourse.tile import TileContext

    f32 = mybir.dt.float32
    # bf16: full-speed PE path AND fast weight load (FWL reads 2 bf16
    # weights/cycle, fp32 gets no FWL win) — fp32r LDWEIGHTS (~187ns) gates
    # the LDW+MM pair at ~120ns vs the 106.7ns matmul stream at FD=256.
    # fp32r: full-speed PE path (1 cyc/row vs 4 for fp32), slow LDW.
    if MM_DT == "bf16":
        mdt = mybir.dt.bfloat16
    elif MM_DT == "fp32r":
        mdt = mybir.dt.float32r
    else:
        mdt = f32
    odt = mybir.dt.bfloat16 if MM_DT == "bf16" else f32
    Sig = mybir.ActivationFunctionType.Sigmoid
    Tanh = mybir.ActivationFunctionType.Tanh

    nc = bacc.Bacc("TRN2", target_bir_lowering=False, debug=False, num_devices=NC)

    # x pre-arranged on host as [T, p(128), k(KT), b(NB)] so each step's
    # load is one fully-contiguous 2D DMA (2KB/partition bursts). The
    # previous [T, D, NB] + on-chip rearrange produced 512B-strided
    # packets that arrived too late and stalled the PE ~0.4us/step.
    xt_d = nc.dram_tensor("xt", [T, 128, KT * NB], mdt, kind="ExternalInput")
    w_d = {}
    for wname in ("wzx", "wzh", "wrx", "wrh", "whx", "whh"):
        w_d[wname] = nc.dram_tensor(wname, [128, KT * MT * 128], mdt, kind="ExternalInput")
    b_d = {}
    for bname in ("bz", "br", "bh"):
        b_d[bname] = nc.dram_tensor(bname, [128, MT], f32, kind="ExternalInput")
    out_d = nc.dram_tensor("out", [T, D, NB], odt, kind="ExternalOutput")

    with TileContext(nc) as tc:
        with (
            tc.tile_pool(name="wsb", bufs=1) as wsb,
            tc.tile_pool(name="xsb", bufs=4) as xsb,
            tc.tile_pool(name="ssb", bufs=2) as ssb,
            tc.tile_pool(name="hsb", bufs=3) as hsb,
            tc.tile_pool(name="psum", bufs=1, space="PSUM") as psum,
        ):
            w_sb = {}
            for wname in w_d:
                w_sb[wname] = wsb.tile([128, KT * MT * 128], mdt, name=f"w_{wname}")
            b_sb = {}
            for bname in b_d:
                b_sb[bname] = wsb.tile([128, MT], f32, name=f"b_{bname}")

            from concourse.tile import add_dep_helper

            def wdma(wname, nchunks=4):
                # chunked so the pieces spread across DMA queues
                insts = []
                cw = KT * MT * 128 // nchunks
                for u in range(nchunks):
                    insts.append(
                        nc.sync.dma_start(out=w_sb[wname][:, u * cw:(u + 1) * cw],
                                          in_=w_d[wname][:, u * cw:(u + 1) * cw]))
                return insts

            # priority set: everything t=0's first matmuls need. All DMA
            # queues round-robin, so the late weight DMAs are gated on the
            # priority set's completion to give it the full HBM bandwidth.
            pri = []
            pri += wdma("wzx")
            pri += wdma("whx")
            pri.append(nc.sync.dma_start(out=b_sb["bz"][:], in_=b_d["bz"][:]))
            pri.append(nc.sync.dma_start(out=b_sb["bh"][:], in_=b_d["bh"][:]))
            xt0 = xsb.tile([128, KT * NB], mdt, name="x0", tag="xt")
            pri.append(nc.sync.dma_start(out=xt0[:], in_=xt_d[0]))

            late = []
            late += wdma("wzh")
            late += wdma("whh")
            late += wdma("wrx")
            late += wdma("wrh")
            late.append(nc.sync.dma_start(out=b_sb["br"][:], in_=b_d["br"][:]))
            pri_gate = (pri[3], pri[7], pri[-1])  # last wzx/whx chunks + xt0
            for li in late:
                for pi in pri_gate:
                    add_dep_helper(li.ins, pi.ins, sync=True,
                                   reason="startup DMA priority")

            # PE pre-warm: HAM leaves the PE at 1.2 GHz until ~3.4us of
            # sustained activity. Dummy matmuls on a memset tile keep the
            # PE busy through the startup DMA window so t=0 runs at 2.4
            # GHz; later bursts are paced by weight-chunk arrivals so the
            # MID window never sees >3.4us of PE idle.
            warm_w = wsb.tile([128, 128], mdt, name="warm_w")
            nc.vector.memset(warm_w[:], 0.0)
            warm_ps = psum.tile([128, 512], f32, name="warm_ps", tag="warm")
            for i in range(48):
                nc.tensor.matmul(warm_ps[:, :64], warm_w[:], warm_w[:, :64],
                                 start=True, stop=True)
            for gate_on in (pri[1], pri[3], pri[5], pri[7]):
                for i in range(6):
                    wm = nc.tensor.matmul(warm_ps[:, :64], warm_w[:],
                                          warm_w[:, :64], start=True, stop=True)
                    add_dep_helper(wm.ins, gate_on.ins, sync=True,
                                   reason="paced PE warmup")

            def wtile(wname, k, mi):
                off = (k * MT + mi) * 128
                return w_sb[wname][:, off:off + 128]

            def hview(h_m):
                # DVE can read bf16 directly; float32r tiles need a f32
                # bitcast for DVE/ACT consumption.
                return h_m[:] if MM_DT == "bf16" else h_m[:].bitcast(f32)

            h_prev = [None] * MT

            for t in range(T):
                if t == 0:
                    xt_t = xt0
                else:
                    xt_t = xsb.tile([128, KT * NB], mdt, name=f"x{t}", tag="xt")
                    xdma = nc.sync.dma_start(out=xt_t[:], in_=xt_d[t])
                    if t <= 3:
                        # don't let prefetch steal HBM BW from the startup
                        # priority set
                        for pi in pri_gate:
                            add_dep_helper(xdma.ins, pi.ins, sync=True,
                                           reason="startup DMA priority")

                def xts(k):
                    return xt_t[:, k * NB:(k + 1) * NB]

                # --- PSUM banks: z01,z23,r01,r23,g01,g23 (2 m-tiles per bank)
                zb = [psum.tile([128, 512], f32, name=f"z{t}_{i}", tag=f"zb{i}") for i in range(2)]
                gb = [psum.tile([128, 512], f32, name=f"g{t}_{i}", tag=f"gb{i}") for i in range(2)]
                if t > 0:
                    rb = [psum.tile([128, 512], f32, name=f"r{t}_{i}", tag=f"rb{i}") for i in range(2)]

                def half(banks, mi):
                    return banks[mi // 2][:, (mi % 2) * NB:(mi % 2 + 1) * NB]

                # PSUM accumulation groups are tracked per BANK (zero
                # region): exactly one start=True (first MM into the bank)
                # and one stop=True (last MM into the bank) even though the
                # two m-halves are separate output regions.

                # --- x-projections (no recurrence dependency)
                for mi in range(MT):
                    for k in range(KT):
                        nc.tensor.matmul(half(zb, mi), wtile("wzx", k, mi), xts(k),
                                         start=(mi % 2 == 0 and k == 0),
                                         stop=(t == 0 and mi % 2 == 1 and k == KT - 1))
                if t > 0:
                    for mi in range(MT):
                        for k in range(KT):
                            nc.tensor.matmul(half(rb, mi), wtile("wrx", k, mi), xts(k),
                                             start=(mi % 2 == 0 and k == 0), stop=False)
                for mi in range(MT):
                    for k in range(KT):
                        nc.tensor.matmul(half(gb, mi), wtile("whx", k, mi), xts(k),
                                         start=(mi % 2 == 0 and k == 0),
                                         stop=(t == 0 and mi % 2 == 1 and k == KT - 1))

                # --- recurrent parts
                if t > 0:
                    for mi in range(MT):
                        for k in range(KT):
                            nc.tensor.matmul(half(zb, mi), wtile("wzh", k, mi), h_prev[k][:],
                                             start=False,
                                             stop=(mi % 2 == 1 and k == KT - 1))
                    for mi in range(MT):
                        for k in range(KT):
                            nc.tensor.matmul(half(rb, mi), wtile("wrh", k, mi), h_prev[k][:],
                                             start=False,
                                             stop=(mi % 2 == 1 and k == KT - 1))

                    # r gate first (feeds r*h -> Whh matmuls)
                    r_t, rh_t = [], []
                    for mi in range(MT):
                        r_m = ssb.tile([128, NB], f32, name=f"r{t}m{mi}", tag=f"r{mi}")
                        nc.scalar.activation(r_m[:], half(rb, mi), Sig, bias=b_sb["br"][:, mi:mi + 1])
                        r_t.append(r_m)
                    for mi in range(MT):
                        rh_m = ssb.tile([128, NB], mdt, name=f"rh{t}m{mi}", tag=f"rh{mi}")
                        nc.vector.tensor_mul(rh_m[:], r_t[mi][:], hview(h_prev[mi]))
                        rh_t.append(rh_m)

                    for mi in range(MT):
                        for k in range(KT):
                            nc.tensor.matmul(half(gb, mi), wtile("whh", k, mi), rh_t[k][:],
                                             start=False,
                                             stop=(mi % 2 == 1 and k == KT - 1))

                # --- gates and blend
                z_t, g_t, h_t = [], [], []
                for mi in range(MT):
                    z_m = ssb.tile([128, NB], f32, name=f"z{t}m{mi}", tag=f"z{mi}")
                    nc.scalar.activation(z_m[:], half(zb, mi), Sig, bias=b_sb["bz"][:, mi:mi + 1])
                    z_t.append(z_m)
                for mi in range(MT):
                    g_m = ssb.tile([128, NB], f32, name=f"g{t}m{mi}", tag=f"g{mi}")
                    nc.scalar.activation(g_m[:], half(gb, mi), Tanh, bias=b_sb["bh"][:, mi:mi + 1])
                    g_t.append(g_m)

                for mi in range(MT):
                    h_m = hsb.tile([128, NB], mdt, name=f"h{t}m{mi}", tag=f"h{mi}")
                    tmp = ssb.tile([128, NB], f32, name=f"tmp{t}m{mi}", tag=f"tmp{mi}")
                    if t == 0:
                        # h = (1 - z) * g = g - z*g
                        nc.vector.tensor_mul(tmp[:], z_t[mi][:], g_t[mi][:])
                        nc.vector.tensor_sub(h_m[:], g_t[mi][:], tmp[:])
                    else:
                        # h = g + z*(h_prev - g)
                        nc.vector.tensor_sub(tmp[:], hview(h_prev[mi]), g_t[mi][:])
                        nc.vector.tensor_mul(tmp[:], tmp[:], z_t[mi][:])
                        nc.vector.tensor_add(h_m[:], g_t[mi][:], tmp[:])
                    h_t.append(h_m)
                    nc.sync.dma_start(
                        out=out_d[t, mi * 128:(mi + 1) * 128, :],
                        in_=h_m[:] if MM_DT == "bf16" else h_m[:].bitcast(f32),
                    )
                h_prev = h_t

    nc.compile()
    return nc


def _get_nc():
    key = MM_DT
    if key not in _CACHE:
        _CACHE[key] = _build_nc()
    return _CACHE[key]


def _mm_np_dtype():
    if MM_DT == "bf16":
        import ml_dtypes
        return np.dtype(ml_dtypes.bfloat16)
    return np.dtype(np.float32)


def _pack_w(W):
    # W [dout, din] -> lhsT tiles packed [128, KT*MT*128], tile (k,m) at
    # free offset (k*MT+m)*128: w[p, off+q] = W[m*128+q, k*128+p]
    Wt = np.asarray(W, np.float32).T.reshape(KT, 128, MT, 128)
    packed = np.ascontiguousarray(Wt.transpose(1, 0, 2, 3).reshape(128, KT * MT * 128))
    return packed.astype(_mm_np_dtype())


def kernel(inputss, Wzx, Wzh, Wrx, Wrh, Whx, Whh, bz, br, bh):
    global LAST_RESULT
    from concourse.bass_utils import run_bass_kernel_spmd

    inputss = np.asarray(inputss, np.float32)
    assert inputss.shape == (N, T, D), inputss.shape

    # host-side shard + layout prep: [NC, T, p(128), k(KT), b(NB)] so the
    # per-step device DMA is fully contiguous
    xs = (inputss.reshape(NC, NB, T, KT, 128)
          .transpose(0, 2, 4, 3, 1)
          .reshape(NC, T, 128, KT * NB))
    wp = {"wzx": _pack_w(Wzx), "wzh": _pack_w(Wzh),
          "wrx": _pack_w(Wrx), "wrh": _pack_w(Wrh),
          "whx": _pack_w(Whx), "whh": _pack_w(Whh)}
    bp = {"bz": np.ascontiguousarray(np.asarray(bz, np.float32).reshape(MT, 128).T),
          "br": np.ascontiguousarray(np.asarray(br, np.float32).reshape(MT, 128).T),
          "bh": np.ascontiguousarray(np.asarray(bh, np.float32).reshape(MT, 128).T)}

    mmdt = _mm_np_dtype()
    in_maps = []
    for c in range(NC):
        m = {"xt": np.ascontiguousarray(xs[c]).astype(mmdt)}
        m.update(wp)
        m.update(bp)
        in_maps.append(m)

    nc = _get_nc()
    trace = bool(int(os.environ.get("GRU_TRACE", "0")))
    res = run_bass_kernel_spmd(nc, in_maps, core_ids=list(range(NC)), trace=trace)
    LAST_RESULT = res

    outs = np.stack([np.asarray(res.results[c]["out"], np.float32)
                     for c in range(NC)])  # [NC, T, D, NB]
    return np.ascontiguousarray(outs.transpose(0, 3, 1, 2).reshape(N, T, D))

